# revision 25
# baseline (speedup 1.0000x reference)
"""PixPro loss kernel for 8 Trainium2 NeuronCores.

Data-parallel over batch: 1024 samples -> 128 per core (one SBUF
partition per sample for the mask/tail math).

Final architecture (3 compute engines + DMA, fp16 features):
  Host casts features to fp16 and lays them out channel-major
  [128 c_lo, 4 c_hi, 49 n, 128 b] per core -> HBM traffic halves to
  12.8MB/core (~31us at the measured ~420 GB/s stream rate).

  - DMA: params packed into one [128,17] tensor leading the sync ring;
    all 16 feature half-slices interleaved (bt,mt) on the SAME sync
    HWDGE ring (one ring drives all 16 SDMA engines at line rate and
    keeps the ACT instruction queue free of DMA triggers). Chunk 3 is
    DMA'd in quarters so the pipeline drains quickly.
  - DVE: products b*m (fp16 tensor_tensor ~2x mode), m^2 for chunks
    0 and 3, both 0/1 masks as ONE fused broadcast op each
    ((-(DX2-tau2)) is_gt DY2 <=> D2 < tau2, accum -> nnz), the mask
    marginal reduces (two-step, layouts chosen so step A reads
    contiguously), PSUM accumulation, tail assembly.
  - ACT: b^2 squares (per half-chunk) + m^2 for chunks 1-2, sqrt.
    A dummy sqrt up front makes walrus load `sqrt_and_others` (which
    also holds Square/Sign/Copy) so no mid-kernel table switch.
  - PE:  all 3 quantities x 49 points x 4 chunks as ones-rhs matmuls
    (lhsT = scratch slice [128c x 128b], rhs = ones -> psum column
    [128 b, 1]); ~35ns per ldweights+matmul pair; results land
    directly in [batch-partition, point] layout. One PSUM tile per
    chunk, single-shot accumulation groups.
  - GPSIMD is used ONLY for nothing-compute (it shares an SBUF port
    pair with DVE and fully blocks dual-read DVE ops).

Masks never materialize D2: colsum_b/rowsum_m come from two
tensor_reduce steps per side; s = sum(cos * marginal) and
nnz = mask-op accum.

Per-core output [128, 2] = (masked loss contribution, intersection
flag); host does the final psum + divide (the sharding_hint reduction).
"""

import sys

import numpy as np

if "/opt/trn_rl_repo" not in sys.path:
    sys.path.insert(0, "/opt/trn_rl_repo")

B = 1024
C = 512
S = 7
N = S * S  # 49
NCORES = 8
BP = B // NCORES  # 128 samples per core
NCHUNK = 4  # channel chunks of 128
EPS = 1e-6
THRESH2 = 0.7 * 0.7
NTOT = N * N  # 2401

HALVES = [(0, 25), (25, 49)]
# m^2 ownership: 'v' DVE, 'a' ACT -- per (chunk, half)
M2_OWN = {
    (0, 0): "v", (0, 1): "v", (1, 0): "a", (1, 1): "a",
    (2, 0): "a", (2, 1): "a", (3, 0): "v", (3, 1): "v",
}

_t7 = np.linspace(0.0, 1.0, S).astype(np.float32)
T7_TAB = np.ascontiguousarray(np.tile(_t7, (BP, 1)))  # [128, 7]

_NC = None


def _emit(tc, d):
    from contextlib import ExitStack

    from concourse import mybir

    nc = tc.nc
    f32 = mybir.dt.float32
    f16 = mybir.dt.float16
    A = mybir.AluOpType
    AX = mybir.AxisListType
    ACTF = mybir.ActivationFunctionType

    with ExitStack() as ctx:
        pers = ctx.enter_context(tc.tile_pool(name="pers", bufs=1))
        io = ctx.enter_context(tc.tile_pool(name="io", bufs=4))
        ps_pool = ctx.enter_context(tc.tile_pool(name="ps", bufs=1, space="PSUM"))

        # ---- persistent tiles ----
        bt = pers.tile([BP, NCHUNK, N, BP], f16, tag="bt")
        mt = pers.tile([BP, NCHUNK, N, BP], f16, tag="mt")
        par_t = pers.tile([BP, 17], f32, tag="par_t")
        pb_t = par_t[:, 0:4]
        pm_t = par_t[:, 4:8]
        fb_t = par_t[:, 8:9]
        fm_t = par_t[:, 9:10]
        t7_t = par_t[:, 10:17]
        ones = pers.tile([BP, 1], f16, tag="ones")
        psums = []
        for h in range(NCHUNK):
            psum_h = ps_pool.tile([BP, 3 * N], f32, tag=f"psum{h}")
            psums.append(psum_h)

        # packed param DMA leads the sync ring (lands ~2us after trigger)
        nc.sync.dma_start(par_t[:], d["par"][:])
        nc.vector.memset(ones[:], 1.0)

        # dummy sqrt first so walrus picks `sqrt_and_others` (also has
        # Square/Copy/Identity) -> exactly one ACT table load
        dummy = pers.tile([BP, 1], f32, tag="dummy")
        nc.vector.memset(dummy[:], 1.0)
        nc.scalar.sqrt(dummy[:], dummy[:])

        # feature DMAs: all 16 interleaved halves on the sync ring --
        # one HWDGE ring drives all 16 SDMA engines at line rate, and
        # keeps the ACT instruction queue free of DMA triggers
        def dma_slices(h):
            if h < NCHUNK - 1:
                return HALVES
            return [(0, 13), (13, 25), (25, 37), (37, 49)]

        for h in range(NCHUNK):
            for lo, hi in dma_slices(h):
                nc.sync.dma_start(bt[:, h, lo:hi, :], d["bt"][:, h, lo:hi, :])
                nc.sync.dma_start(mt[:, h, lo:hi, :], d["mt"][:, h, lo:hi, :])

        xb = pb_t[:, 0:1]
        yb = pb_t[:, 1:2]
        wb = pb_t[:, 2:3]
        hb = pb_t[:, 3:4]
        xm = pm_t[:, 0:1]
        ym = pm_t[:, 1:2]
        wm = pm_t[:, 2:3]
        hm = pm_t[:, 3:4]

        # ---- mask geometry (small DVE ops during the DMA stream) ----
        yb2 = pers.tile([BP, 1], f32, tag="yb2")
        hb2 = pers.tile([BP, 1], f32, tag="hb2")
        ym2 = pers.tile([BP, 1], f32, tag="ym2")
        hm2 = pers.tile([BP, 1], f32, tag="hm2")
        tmp1 = pers.tile([BP, 1], f32, tag="tmp1")
        tmp2 = pers.tile([BP, 1], f32, tag="tmp2")
        nc.vector.scalar_tensor_tensor(yb2[:], fb_t, hb, yb, A.mult, A.add)
        nc.vector.tensor_scalar(tmp1[:], fb_t, -2.0, 1.0, A.mult, A.add)
        nc.vector.tensor_tensor(hb2[:], tmp1[:], hb, A.mult)
        nc.vector.scalar_tensor_tensor(ym2[:], fm_t, hm, ym, A.mult, A.add)
        nc.vector.tensor_scalar(tmp1[:], fm_t, -2.0, 1.0, A.mult, A.add)
        nc.vector.tensor_tensor(hm2[:], tmp1[:], hm, A.mult)

        bx = pers.tile([BP, S], f32, tag="bx")
        by = pers.tile([BP, S], f32, tag="by")
        mx = pers.tile([BP, S], f32, tag="mx")
        my = pers.tile([BP, S], f32, tag="my")
        nc.vector.tensor_scalar(bx[:], t7_t, wb, xb, A.mult, A.add)
        nc.vector.tensor_scalar(by[:], t7_t, hb2[:, 0:1], yb2[:, 0:1], A.mult, A.add)
        nc.vector.tensor_scalar(mx[:], t7_t, wm, xm, A.mult, A.add)
        nc.vector.tensor_scalar(my[:], t7_t, hm2[:, 0:1], ym2[:, 0:1], A.mult, A.add)

        dx = pers.tile([BP, S, S], f32, tag="dx")
        dy = pers.tile([BP, S, S], f32, tag="dy")
        dx2 = pers.tile([BP, S, S], f32, tag="dx2")
        dy2 = pers.tile([BP, S, S], f32, tag="dy2")
        nc.vector.tensor_tensor(
            dx[:], bx[:].unsqueeze(2).broadcast_to([BP, S, S]),
            mx[:].unsqueeze(1).broadcast_to([BP, S, S]), A.subtract,
        )
        nc.vector.tensor_tensor(
            dy[:], by[:].unsqueeze(2).broadcast_to([BP, S, S]),
            my[:].unsqueeze(1).broadcast_to([BP, S, S]), A.subtract,
        )
        nc.vector.tensor_tensor(dx2[:], dx[:], dx[:], A.mult)
        nc.vector.tensor_tensor(dy2[:], dy[:], dy[:], A.mult)

        tau2b = pers.tile([BP, 1], f32, tag="tau2b")
        tau2m = pers.tile([BP, 1], f32, tag="tau2m")
        nc.vector.tensor_tensor(tmp1[:], wb, wb, A.mult)
        nc.vector.scalar_tensor_tensor(tau2b[:], hb, hb, tmp1[:], A.mult, A.add)
        nc.vector.tensor_scalar_mul(tau2b[:], tau2b[:], THRESH2)
        nc.vector.tensor_tensor(tmp1[:], wm, wm, A.mult)
        nc.vector.scalar_tensor_tensor(tau2m[:], hm, hm, tmp1[:], A.mult, A.add)
        nc.vector.tensor_scalar_mul(tau2m[:], tau2m[:], THRESH2)

        # ---- tau^2-folded axis distances; masks fused on DVE ----
        # b-side in layout [(i',i),(j',j)]; m-side in [(i,i'),(j,j')].
        # Both make the first marginal reduce read contiguously.
        dx2b = pers.tile([BP, S, S], f32, tag="dx2b")
        dx2m = pers.tile([BP, S, S], f32, tag="dx2m")
        dx2bT = pers.tile([BP, N], f32, tag="dx2bT")
        dy2T = pers.tile([BP, N], f32, tag="dy2T")
        nc.vector.tensor_scalar(dx2b[:], dx2[:], tau2b[:, 0:1], None, A.subtract)
        nc.vector.tensor_scalar(dx2m[:], dx2[:], tau2m[:, 0:1], None, A.subtract)
        nc.vector.tensor_copy(
            dx2bT[:].rearrange("p (a b) -> p a b", a=S), dx2b[:].transpose([0, 2, 1])
        )
        nc.vector.tensor_copy(
            dy2T[:].rearrange("p (a b) -> p a b", a=S), dy2[:].transpose([0, 2, 1])
        )
        # masks fused on DVE: mask = (-dx2') is_gt dy2' <=> DX2+DY2 < tau2
        # (no D2 materialization, no GPSIMD -- it interferes with DVE)
        mask_bT = pers.tile([BP, N, N], f16, tag="mask_bT")  # [(i',i),(j',j)]
        mask_m = pers.tile([BP, N, N], f16, tag="mask_m")  # [(i,i'),(j,j')]
        nnzb = pers.tile([BP, 1], f32, tag="nnzb")
        nnzm = pers.tile([BP, 1], f32, tag="nnzm")
        dx2mf = dx2m[:].rearrange("p a b -> p (a b)")
        dy2f = dy2[:].rearrange("p a b -> p (a b)")

        def emit_masks():
            nc.vector.scalar_tensor_tensor(
                mask_bT[:],
                dx2bT[:].unsqueeze(2).broadcast_to([BP, N, N]), -1.0,
                dy2T[:].unsqueeze(1).broadcast_to([BP, N, N]),
                A.mult, A.is_gt, accum_out=nnzb[:],
            )
            nc.vector.scalar_tensor_tensor(
                mask_m[:],
                dx2mf.unsqueeze(2).broadcast_to([BP, N, N]), -1.0,
                dy2f.unsqueeze(1).broadcast_to([BP, N, N]),
                A.mult, A.is_gt, accum_out=nnzm[:],
            )

        # ---- heavy part ----
        red = pers.tile([BP, 3 * N], f32, tag="red")
        rA = pers.tile([BP, N, S], f32, tag="rA")
        rB = pers.tile([BP, N, S], f32, tag="rB")
        scol_b = pers.tile([BP, S, S], f32, tag="scol_b")
        srow_m = pers.tile([BP, S, S], f32, tag="srow_m")

        def emit_reduce_b():
            # colsum_b[(i',j')] = sum_{i,j} mask_bT[(i',i),(j',j)]
            nc.vector.tensor_reduce(
                rA[:], mask_bT[:].rearrange("p a (j k) -> p a j k", j=S), AX.X, A.add
            )  # sum over j -> [p, (i',i), j']
            rA4 = rA[:].rearrange("p (ip i) k -> p ip i k", ip=S)
            nc.vector.tensor_reduce(
                scol_b[:], rA4.transpose([0, 1, 3, 2]), AX.X, A.add
            )  # sum over i -> [p, i', j']

        def emit_reduce_m():
            # rowsum_m[(i,j)] = sum_{i',j'} mask_m[(i,i'),(j,j')]
            nc.vector.tensor_reduce(
                rB[:], mask_m[:].rearrange("p a (j k) -> p a j k", j=S), AX.X, A.add
            )  # sum over j' -> [p, (i,i'), j]
            rB4 = rB[:].rearrange("p (i ip) j -> p i ip j", i=S)
            nc.vector.tensor_reduce(
                srow_m[:], rB4.transpose([0, 1, 3, 2]), AX.X, A.add
            )  # sum over i' -> [p, i, j]

        NH = 25  # padded half size; hf=1 uses [:, :24, :]
        for h in range(NCHUNK):
            for hf, (lo, hi) in enumerate(HALVES):
                nh = hi - lo
                own = M2_OWN[(h, hf)]
                prod_f = io.tile([BP, NH, BP], f16, tag="prod_f")
                sqb_f = io.tile([BP, NH, BP], f16, tag="sqb_f")
                sqm_f = io.tile([BP, NH, BP], f16, tag="sqm_f")
                prod_s, sqb_s, sqm_s = (
                    prod_f[:, 0:nh, :], sqb_f[:, 0:nh, :], sqm_f[:, 0:nh, :]
                )
                if h == NCHUNK - 1:
                    mid = nh // 2
                    for qlo, qhi in ((0, mid), (mid, nh)):
                        nc.vector.tensor_tensor(
                            prod_f[:, qlo:qhi, :],
                            bt[:, h, lo + qlo : lo + qhi, :],
                            mt[:, h, lo + qlo : lo + qhi, :], A.mult,
                        )
                        nc.vector.tensor_tensor(
                            sqm_f[:, qlo:qhi, :],
                            mt[:, h, lo + qlo : lo + qhi, :],
                            mt[:, h, lo + qlo : lo + qhi, :], A.mult,
                        )
                else:
                    nc.vector.tensor_tensor(
                        prod_s, bt[:, h, lo:hi, :], mt[:, h, lo:hi, :], A.mult
                    )
                    if own == "v":
                        nc.vector.tensor_tensor(
                            sqm_s, mt[:, h, lo:hi, :], mt[:, h, lo:hi, :], A.mult
                        )
                    else:
                        nc.scalar.activation(
                            sqm_s.rearrange("p a b -> p (a b)"),
                            mt[:, h, lo:hi, :].rearrange("p a b -> p (a b)"),
                            ACTF.Square,
                        )
                nc.scalar.activation(
                    sqb_s.rearrange("p a b -> p (a b)"),
                    bt[:, h, lo:hi, :].rearrange("p a b -> p (a b)"),
                    ACTF.Square,
                )
                for q, scr in ((0, prod_s), (1, sqb_s), (2, sqm_s)):
                    for j in range(nh):
                        n = lo + j
                        nc.tensor.matmul(
                            psums[h][:, q * N + n : q * N + n + 1],
                            scr[:, j, :],
                            ones[:],
                            start=True,
                            stop=True,
                        )
            if h == 0:
                emit_masks()
            if h == 1:
                emit_reduce_b()
                nc.vector.tensor_copy(red[:], psums[0][:])
                nc.vector.tensor_tensor(red[:], red[:], psums[1][:], A.add)
            if h == 2:
                emit_reduce_m()

        # ---- intersection flag via squares (DVE-only) ----
        u1 = pers.tile([BP, 1], f32, tag="u1")
        u2 = pers.tile([BP, 1], f32, tag="u2")
        okx = pers.tile([BP, 1], f32, tag="okx")
        inter = pers.tile([BP, 1], f32, tag="inter")
        nc.vector.scalar_tensor_tensor(u1[:], wb, 0.5, xb, A.mult, A.add)
        nc.vector.scalar_tensor_tensor(u2[:], wm, 0.5, xm, A.mult, A.add)
        nc.vector.tensor_tensor(u1[:], u1[:], u2[:], A.subtract)
        nc.vector.tensor_tensor(u1[:], u1[:], u1[:], A.mult)
        nc.vector.tensor_tensor(u2[:], wb, wm, A.add)
        nc.vector.tensor_tensor(u2[:], u2[:], u2[:], A.mult)
        nc.vector.scalar_tensor_tensor(okx[:], u1[:], 4.0, u2[:], A.mult, A.is_lt)
        nc.vector.scalar_tensor_tensor(u1[:], hb, 0.5, yb, A.mult, A.add)
        nc.vector.scalar_tensor_tensor(u2[:], hm, 0.5, ym, A.mult, A.add)
        nc.vector.tensor_tensor(u1[:], u1[:], u2[:], A.subtract)
        nc.vector.tensor_tensor(u1[:], u1[:], u1[:], A.mult)
        nc.vector.tensor_tensor(u2[:], hb, hm, A.add)
        nc.vector.tensor_tensor(u2[:], u2[:], u2[:], A.mult)
        nc.vector.scalar_tensor_tensor(tmp2[:], u1[:], 4.0, u2[:], A.mult, A.is_lt)
        nc.vector.tensor_tensor(inter[:], okx[:], tmp2[:], A.mult)

        # ---- tail: finish PSUM accumulation, then cos ----
        nc.vector.tensor_tensor(red[:], red[:], psums[2][:], A.add)
        nc.vector.tensor_tensor(red[:], red[:], psums[3][:], A.add)
        dot = red[:, 0:N]
        b2 = red[:, N : 2 * N]
        m2 = red[:, 2 * N : 3 * N]
        den = pers.tile([BP, N], f32, tag="den")
        cos_t = pers.tile([BP, N], f32, tag="cos_t")
        nc.vector.tensor_tensor(den[:], b2, m2, A.mult)
        nc.scalar.sqrt(den[:], den[:])
        nc.vector.reciprocal(den[:], den[:])
        nc.vector.tensor_tensor(cos_t[:], dot, den[:], A.mult)

        scr_n = pers.tile([BP, N], f32, tag="scr_n")
        sb_s = pers.tile([BP, 1], f32, tag="sb_s")
        sm_s = pers.tile([BP, 1], f32, tag="sm_s")
        nc.vector.scalar_tensor_tensor(
            scr_n[:], cos_t[:], 1.0,
            scol_b[:].rearrange("p a b -> p (a b)"), A.mult, A.mult,
            accum_out=sb_s[:],
        )
        nc.vector.scalar_tensor_tensor(
            scr_n[:], cos_t[:], 1.0,
            srow_m[:].rearrange("p a b -> p (a b)"), A.mult, A.mult,
            accum_out=sm_s[:],
        )
        lb = pers.tile([BP, 1], f32, tag="lb")
        lm = pers.tile([BP, 1], f32, tag="lm")
        out_sb = pers.tile([BP, 2], f32, tag="out_sb")
        nc.vector.tensor_scalar_max(nnzb[:], nnzb[:], 1.0)
        nc.vector.tensor_scalar_max(nnzm[:], nnzm[:], 1.0)
        nc.vector.reciprocal(nnzb[:], nnzb[:])
        nc.vector.reciprocal(nnzm[:], nnzm[:])
        nc.vector.tensor_tensor(lb[:], sb_s[:], nnzb[:], A.mult)
        nc.vector.tensor_tensor(lm[:], sm_s[:], nnzm[:], A.mult)
        nc.vector.tensor_tensor(lb[:], lb[:], lm[:], A.add)
        nc.vector.tensor_tensor(lb[:], lb[:], inter[:], A.mult)
        nc.vector.tensor_copy(out_sb[:, 0:1], lb[:])
        nc.vector.tensor_copy(out_sb[:, 1:2], inter[:])

        nc.sync.dma_start(d["o"][:], out_sb[:])


def build(debug=False):
    import concourse.bacc as bacc
    import concourse.tile as tile
    from concourse import mybir

    nc = bacc.Bacc(
        "TRN2",
        target_bir_lowering=False,
        debug=debug,
        enable_asserts=False,
        num_devices=NCORES,
    )
    f32 = mybir.dt.float32
    f16 = mybir.dt.float16
    d = {
        "bt": nc.dram_tensor("bt", [BP, NCHUNK, N, BP], f16, kind="ExternalInput").ap(),
        "mt": nc.dram_tensor("mt", [BP, NCHUNK, N, BP], f16, kind="ExternalInput").ap(),
        "par": nc.dram_tensor("par", [BP, 17], f32, kind="ExternalInput").ap(),
        "o": nc.dram_tensor("o", [BP, 2], f32, kind="ExternalOutput").ap(),
    }
    with tile.TileContext(nc) as tc:
        _emit(tc, d)
    nc.compile()
    return nc


def _cm(feat_core):
    """[BP, C, N] f32 -> channel-major fp16 [128 c_lo, 4 c_hi, N, 128 b]."""
    a = feat_core.reshape(BP, C, N).transpose(1, 2, 0)  # [C, N, B]
    a = a.reshape(NCHUNK, 128, N, BP).transpose(1, 0, 2, 3)  # [c_lo, c_hi, N, B]
    return np.ascontiguousarray(a.astype(np.float16))


def make_in_maps(base, moment, p_base, p_moment, f_base, f_moment):
    base = np.asarray(base, dtype=np.float32)
    moment = np.asarray(moment, dtype=np.float32)
    in_maps = []
    for k in range(NCORES):
        sl = slice(k * BP, (k + 1) * BP)
        par = np.concatenate(
            [
                np.asarray(p_base[sl], dtype=np.float32),
                np.asarray(p_moment[sl], dtype=np.float32),
                np.asarray(f_base[sl], dtype=np.float32),
                np.asarray(f_moment[sl], dtype=np.float32),
                T7_TAB,
            ],
            axis=1,
        )
        in_maps.append(
            {
                "bt": _cm(base[sl]),
                "mt": _cm(moment[sl]),
                "par": np.ascontiguousarray(par),
            }
        )
    return in_maps


def reduce_outputs(per_core_outs):
    allo = np.concatenate([np.asarray(o, dtype=np.float64) for o in per_core_outs])
    pos = allo[:, 0].sum()
    cnt = allo[:, 1].sum()
    return np.asarray(-pos / max(cnt, 1.0), dtype=np.float32)


def kernel(base, moment, p_base, p_moment, f_base, f_moment, _trace=False):
    global _NC
    from concourse.bass_utils import run_bass_kernel_spmd

    if _NC is None:
        _NC = build()
    in_maps = make_in_maps(base, moment, p_base, p_moment, f_base, f_moment)
    res = run_bass_kernel_spmd(_NC, in_maps, core_ids=list(range(NCORES)), trace=_trace)
    out = reduce_outputs([r["o"] for r in res.results])
    if _trace:
        return out, res
    return out
